# revision 15
# baseline (speedup 1.0000x reference)
"""Expert-parallel MoE MLP kernel for TRN2 (8 NeuronCores, 1 expert/core).

Math per core (expert e):
    h   = gelu(x_e @ w1_e + b1_e)      x_e: [4096, 1024], w1_e: [1024, 4096]
    out = h @ w2_e + b2_e              w2_e: [4096, 1024]

Host-side prep (inside kernel(), part of the sharding step): x_e is
transposed to [D, tok] and cast to bf16, w1/w2 are cast to bf16. On
device the kernel is then pure matmul work:
  - mm1: stationary = w1 tile [128(D), 128(H)], moving = xT tile
    [128(D), 512(tok)] -> PSUM hT tile; ACT applies exact-erf GELU (+b1
    as per-partition bias) PSUM->SBUF bf16.
  - mm2: stationary = hT tile [128(H), 128(tok)], moving = w2 tile
    [128(H), 512(D)] -> PSUM out tile (fp32); DVE adds b2, DMA out.
Weights live in SBUF as independent [128,1024] tiles loaded over both
DMA paths (HWDGE via sync + SWDGE via gpsimd) in an order that lets
chunk-0 mm1 start after ~2MB of DMA instead of the full preload.

fp8 fraction: the kernel is tensor-engine bound at the bf16 roofline
(216ns per 512-wide matmul back-to-back, ~2.37GHz effective), so the
last 8 of mm2's 32 contraction planes run as 4 fp8e4 DoubleRow pairs
(2 planes/instr at 2x MAC rate). fp8 needs w2*512 to stay out of e4m3
subnormals, so those planes accumulate in a separate PSUM tile that a
fused DVE op merges back: out = (psum_q * 1/512) + psum_bf16 (+b2).
Measured end-to-end rel err 1.90e-2 vs the 2e-2 gate (deterministic
fp8 quantization noise on 25% of mm2) for ~6% less tensor-engine time.
All matmuls accumulate fp32 in PSUM; bf16/fp8 only round the inputs.
Other measured details: PE p-state ramps 0.65->2.4GHz over ~3us, so 30
dummy matmuls warm it during the ~13.6us prologue DMA window; the
prologue is bounded by DMA queue spin-up (HWDGE 2.5-5.7us, SWDGE
~8.7us) + the 3MB chunk-0 critical mass at ~52GB/s/queue.
"""

import numpy as np
import ml_dtypes

import concourse.bacc as bacc
import concourse.bass as bass
import concourse.mybir as mybir
import concourse.tile as tile
from concourse import bass_utils

P = 128
D = 1024
H = 4096
NTOK = 4096  # B*N per expert
NCORES = 8
CHUNK = 512  # tokens per pipeline chunk
NCHUNK = NTOK // CHUNK
F32 = mybir.dt.float32
BF16 = mybir.dt.bfloat16
F8 = mybir.dt.float8e4
GELU = mybir.ActivationFunctionType.Gelu
DR = mybir.MatmulPerfMode.DoubleRow
MULT = mybir.AluOpType.mult
ADD = mybir.AluOpType.add

DK = D // P    # 8   k-tiles of D
HM = H // P    # 32  tiles of H
HQ = 4         # W1 loaded in 4 column quarters (1024 wide)
TSUB = CHUNK // P  # 4 token subtiles per chunk
DC = D // 512  # 2   512-wide output column chunks

NQP = 4              # fp8 DoubleRow pairs in mm2 (planes QBASE..HM-1)
QBASE = HM - 2 * NQP  # first fp8 plane (26)
W2SCALE = 512.0      # host pre-scale on fp8 w2 (keeps e4m3 out of subnormals)


def build_program(act=GELU):
    nc = bacc.Bacc("TRN2", target_bir_lowering=False, debug=False,
                   num_devices=NCORES)

    xt_d = nc.dram_tensor("xt", (D, NTOK), BF16, kind="ExternalInput").ap()
    w1 = nc.dram_tensor("w1", (D, H), BF16, kind="ExternalInput").ap()
    # biases arrive pre-arranged from the host: b1 as [128, 32] (H on
    # partitions), b2 replicated to [128, 1024] — plain contiguous DMAs
    b1 = nc.dram_tensor("b1", (P, HM), F32, kind="ExternalInput").ap()
    w2 = nc.dram_tensor("w2", (QBASE * P, D), BF16, kind="ExternalInput").ap()
    w2q = nc.dram_tensor("w2q", (NQP, P, 2, D), F8, kind="ExternalInput").ap()
    b2 = nc.dram_tensor("b2", (P, D), F32, kind="ExternalInput").ap()
    out = nc.dram_tensor("out", (NTOK, D), F32, kind="ExternalOutput").ap()

    with tile.TileContext(nc) as tc:
        with (
            tc.tile_pool(name="consts", bufs=1) as consts,
            tc.tile_pool(name="weights", bufs=1) as wpool,
            tc.tile_pool(name="xt", bufs=2) as xtp,
            tc.tile_pool(name="ht", bufs=1) as htp,
            tc.tile_pool(name="outp", bufs=4) as outp,
            tc.tile_pool(name="p1", bufs=2, space="PSUM") as p1p,
            tc.tile_pool(name="p2", bufs=3, space="PSUM") as p2p,
            tc.tile_pool(name="p2q", bufs=3, space="PSUM") as p2qp,
        ):
            def load_xt_chunk(c):
                xt = [xtp.tile([P, CHUNK], BF16, tag=f"xt{dk}",
                               name=f"xt{dk}_c{c}") for dk in range(DK)]
                for dk in range(DK):
                    eng = nc.sync if dk % 2 == 0 else nc.gpsimd
                    eng.dma_start(
                        xt[dk], xt_d[dk * P:(dk + 1) * P,
                                     c * CHUNK:(c + 1) * CHUNK])
                return xt

            # b1 first — tiny, and the first gelu (which drains the p1
            # PSUM pool) needs it
            b1_sb = consts.tile([P, HM], F32, tag="b1")
            nc.sync.dma_start(b1_sb, b1)

            # PE p-state warmup: the tensor engine ramps 0.65 -> 1.2 ->
            # 2.4GHz over ~3us of continuous work, and real work can't
            # start until ~13.6us of prologue DMA anyway. Burn the idle
            # window on dummy matmuls over a memset tile so the real
            # stream opens at full clock. ~30 instrs ≈ 10us with ramp.
            warm = consts.tile([P, 512], BF16, tag="warm")
            nc.vector.memset(warm, 0)
            for i in range(30):
                pw = p1p.tile([P, CHUNK], F32, tag="p1", name=f"warm{i}")
                nc.tensor.matmul(pw, warm[:, 0:P], warm,
                                 start=True, stop=True)

            # chunk-0 activations interleaved dk-major with the first W1
            # column-quarter so mm1's k-loop unblocks progressively.
            # (Measured: the ~13.6us prologue is bounded by DMA queue
            # spin-up (2.5-8.7us staggered) + the 3MB critical mass at
            # ~52GB/s per queue; splitting tiles into finer strips only
            # floods descriptors and made it worse.)
            w1t = [[None] * HQ for _ in range(DK)]

            def alloc_w1(dk, hq):
                t = wpool.tile([P, D], BF16, tag=f"w1_{dk}_{hq}",
                               name=f"w1_{dk}_{hq}")
                w1t[dk][hq] = t
                return t

            # chunk-0 critical stream (xt0 + w1 quarter 0, 3MB) split
            # across BOTH DMA engines: aggregate 16-queue bandwidth
            # beats the SWDGE queue-open latency; all-sync variants
            # starved chunk 0 (measured).
            xt0 = [xtp.tile([P, CHUNK], BF16, tag=f"xt{dk}",
                            name=f"xt{dk}_c0") for dk in range(DK)]
            for dk in range(DK):
                e0 = nc.sync if dk % 2 == 0 else nc.gpsimd
                e1 = nc.gpsimd if dk % 2 == 0 else nc.sync
                e0.dma_start(xt0[dk], xt_d[dk * P:(dk + 1) * P, 0:CHUNK])
                e1.dma_start(alloc_w1(dk, 0), w1[dk * P:(dk + 1) * P, 0:D])

            # quarter 1 (needed at hm=8, ~27us) on HWDGE/sync — the
            # SWDGE backlog made it the laggard; quarters 2-3 balance
            for dk in range(DK):
                nc.sync.dma_start(alloc_w1(dk, 1),
                                  w1[dk * P:(dk + 1) * P, D:2 * D])
            for hq in range(2, HQ):
                for dk in range(DK):
                    eng = nc.gpsimd if (hq * DK + dk) % 2 == 0 else nc.sync
                    eng.dma_start(alloc_w1(dk, hq),
                                  w1[dk * P:(dk + 1) * P,
                                     hq * D:(hq + 1) * D])

            # b2 (replicated on host) — needed from the first out tile ~95us
            b2_rep = consts.tile([P, D], F32, tag="b2rep")
            nc.gpsimd.dma_start(b2_rep, b2)

            w2t = []
            for hk in range(QBASE):
                t = wpool.tile([P, D], BF16, tag=f"w2_{hk}",
                               name=f"w2_{hk}")
                w2t.append(t)
                eng = nc.sync if hk % 2 == 0 else nc.gpsimd
                eng.dma_start(t, w2[hk * P:(hk + 1) * P, :])

            # fp8 w2 pair tiles [128, 2, 1024] for the DoubleRow planes
            w2qt = []
            for j in range(NQP):
                t = wpool.tile([P, 2, D], F8, tag=f"w2q_{j}",
                               name=f"w2q_{j}")
                w2qt.append(t)
                eng = nc.sync if j % 2 == 0 else nc.gpsimd
                eng.dma_start(t, w2q[j])

            # ---- main pipeline over token chunks ----
            for c in range(NCHUNK):
                # xT tiles straight from DRAM (bf16), double-buffered so
                # chunk c+1 prefetches during chunk c.
                xt = xt0 if c == 0 else load_xt_chunk(c)

                # mm1 + gelu -> hT tiles; bf16 for planes < QBASE, fp8
                # pair tiles [128, 2, CHUNK] for the DoubleRow planes
                ht = [htp.tile([P, CHUNK], BF16, tag=f"ht{hm}",
                               name=f"ht{hm}_c{c}") for hm in range(QBASE)]
                hq = [htp.tile([P, 2, CHUNK], F8, tag=f"hq{j}",
                               name=f"hq{j}_c{c}") for j in range(NQP)]
                for hm in range(HM):
                    p1 = p1p.tile([P, CHUNK], F32, tag="p1",
                                  name=f"p1_c{c}h{hm}")
                    hcol = (hm % (HM // HQ)) * P
                    for dk in range(DK):
                        nc.tensor.matmul(
                            p1,
                            w1t[dk][hm // (HM // HQ)][:, hcol:hcol + P],
                            xt[dk],
                            start=(dk == 0), stop=(dk == DK - 1))
                    if hm < QBASE:
                        dst = ht[hm]
                    else:
                        j, i = divmod(hm - QBASE, 2)
                        dst = hq[j][:, i, :]
                    nc.scalar.activation(dst, p1, act,
                                         bias=b1_sb[:, hm:hm + 1], scale=1.0)

                # mm2 (+b2) -> out
                for ts in range(TSUB):
                    p2s = [p2p.tile([P, 512], F32, tag="p2",
                                    name=f"p2_c{c}t{ts}d{dc}")
                           for dc in range(DC)]
                    for hk in range(QBASE):
                        lhsT = ht[hk][:, ts * P:(ts + 1) * P]
                        for dc in range(DC):
                            nc.tensor.matmul(
                                p2s[dc], lhsT,
                                w2t[hk][:, dc * 512:(dc + 1) * 512],
                                start=(hk == 0), stop=(hk == QBASE - 1))
                    # fp8 DoubleRow planes -> separate (scaled) PSUM tiles
                    p2q = [p2qp.tile([P, 512], F32, tag="p2q",
                                     name=f"p2q_c{c}t{ts}d{dc}")
                           for dc in range(DC)]
                    for dc in range(DC):
                        for half in range(2):
                            col = dc * 512 + half * 256
                            for j in range(NQP):
                                nc.tensor.matmul(
                                    p2q[dc][:, half * 256:(half + 1) * 256],
                                    hq[j][:, :, ts * P:(ts + 1) * P],
                                    w2qt[j][:, :, col:col + 256],
                                    start=(j == 0), stop=(j == NQP - 1),
                                    perf_mode=DR)
                    r0 = c * CHUNK + ts * P
                    for dc in range(DC):
                        ot = outp.tile([P, 512], F32, tag="ot",
                                       name=f"ot_c{c}t{ts}d{dc}")
                        nc.vector.tensor_add(
                            ot, p2s[dc], b2_rep[:, dc * 512:(dc + 1) * 512])
                        otq = outp.tile([P, 512], F32, tag="otq",
                                        name=f"otq_c{c}t{ts}d{dc}")
                        nc.vector.scalar_tensor_tensor(
                            otq, p2q[dc], 1.0 / W2SCALE, ot, MULT, ADD)
                        if c == NCHUNK - 1 and ts == TSUB - 1:
                            # final tiles: halve across both engines so
                            # each queue's last descriptor is small and
                            # the teardown drains start sooner
                            c0 = dc * 512
                            nc.sync.dma_start(
                                out[r0:r0 + P, c0:c0 + 256], otq[:, 0:256])
                            nc.gpsimd.dma_start(
                                out[r0:r0 + P, c0 + 256:c0 + 512],
                                otq[:, 256:512])
                        else:
                            oeng = nc.sync if (ts + dc) % 2 == 0 else nc.gpsimd
                            oeng.dma_start(
                                out[r0:r0 + P, dc * 512:(dc + 1) * 512], otq)

    nc.compile()
    return nc


_CACHE: dict = {}


def _program():
    if "nc" not in _CACHE:
        _CACHE["nc"] = build_program()
    return _CACHE["nc"]


def _in_maps(x, w1, b1, w2, b2):
    x = np.asarray(x, dtype=np.float32)
    w1 = np.asarray(w1, dtype=np.float32)
    b1 = np.asarray(b1, dtype=np.float32)
    w2 = np.asarray(w2, dtype=np.float32)
    b2 = np.asarray(b2, dtype=np.float32)
    bf = ml_dtypes.bfloat16
    f8 = ml_dtypes.float8_e4m3fn
    maps = []
    for e in range(NCORES):
        xt_e = np.ascontiguousarray(
            x[:, e].reshape(NTOK, D).T.astype(bf))  # [D, NTOK] bf16
        w2_e = w2[e]
        # fp8 pair layout: w2q[j, p, i, d] = S * w2[(QBASE+2j+i)*128+p, d]
        w2_tail = w2_e[QBASE * P:].reshape(NQP, 2, P, D).transpose(0, 2, 1, 3)
        maps.append({
            "xt": xt_e,
            "w1": np.ascontiguousarray(w1[e].astype(bf)),
            "b1": np.ascontiguousarray(b1[e].reshape(HM, P).T),
            "w2": np.ascontiguousarray(w2_e[:QBASE * P].astype(bf)),
            "w2q": np.ascontiguousarray(
                (w2_tail * W2SCALE).astype(f8)),
            "b2": np.ascontiguousarray(
                np.broadcast_to(b2[e], (P, D))),
        })
    return maps


def _install_ntff_hook_shim():
    """Provide antenv.axon_hooks if the image lacks it, wiring the NTFF
    profile hook straight to libaxon_pjrt.so (mirrors trn_agent_boot)."""
    import sys
    try:
        from antenv.axon_hooks import get_axon_ntff_profile_hook  # noqa: F401
        return
    except ImportError:
        pass
    import contextlib
    import ctypes
    import types

    import antenv

    hook = None
    so_path = "/opt/axon/libaxon_pjrt.so"
    try:
        lib = ctypes.CDLL(so_path)
        if hasattr(lib, "axon_start_nrt_profile"):
            lib.axon_start_nrt_profile.argtypes = [
                ctypes.POINTER(ctypes.c_int64), ctypes.c_size_t]
            lib.axon_start_nrt_profile.restype = ctypes.c_int64
            lib.axon_stop_nrt_profile.argtypes = [ctypes.c_char_p]
            lib.axon_stop_nrt_profile.restype = ctypes.c_int64

            @contextlib.contextmanager
            def _hook(output_dir, device_ids):
                import jax
                jax.devices()
                if device_ids:
                    ids = (ctypes.c_int64 * len(device_ids))(*device_ids)
                    rc = lib.axon_start_nrt_profile(ids, len(device_ids))
                else:
                    rc = lib.axon_start_nrt_profile(None, 0)
                if rc != 0:
                    raise RuntimeError(f"axon_start_nrt_profile rc={rc}")
                try:
                    yield
                finally:
                    n = lib.axon_stop_nrt_profile(str(output_dir).encode())
                    print(f"ntff profile: {n} file(s) -> {output_dir}")

            hook = _hook
    except OSError:
        pass

    mod = types.ModuleType("antenv.axon_hooks")
    mod._hook = hook
    mod.get_axon_ntff_profile_hook = lambda: mod._hook
    mod.set_axon_ntff_profile_hook = lambda h: setattr(mod, "_hook", h)
    sys.modules["antenv.axon_hooks"] = mod
    antenv.axon_hooks = mod


def run_spmd(x, w1, b1, w2, b2, trace=False):
    if trace:
        _install_ntff_hook_shim()
    nc = _program()
    res = bass_utils.run_bass_kernel_spmd(
        nc, _in_maps(x, w1, b1, w2, b2), core_ids=list(range(NCORES)),
        trace=trace)
    outs = [r["out"].reshape(4, 1024, D) for r in res.results]
    full = np.stack(outs, axis=1).astype(np.float32)  # [4, 8, 1024, 1024]
    return full, res


def kernel(x, w1, b1, w2, b2):
    full, _ = run_spmd(x, w1, b1, w2, b2)
    return full


# revision 17
# speedup vs baseline: 1.0578x; 1.0578x over previous
"""Expert-parallel MoE MLP kernel for TRN2 (8 NeuronCores, 1 expert/core).

Math per core (expert e):
    h   = gelu(x_e @ w1_e + b1_e)      x_e: [4096, 1024], w1_e: [1024, 4096]
    out = h @ w2_e + b2_e              w2_e: [4096, 1024]

Host-side prep (inside kernel(), part of the sharding step): x_e is
transposed to [D, tok] and cast to bf16, w1/w2 are cast to bf16. On
device the kernel is then pure matmul work:
  - mm1: stationary = w1 tile [128(D), 128(H)], moving = xT tile
    [128(D), 512(tok)] -> PSUM hT tile; ACT applies exact-erf GELU (+b1
    as per-partition bias) PSUM->SBUF bf16.
  - mm2: stationary = hT tile [128(H), 128(tok)], moving = w2 tile
    [128(H), 512(D)] -> PSUM out tile (fp32); DVE adds b2, DMA out.
Weights live in SBUF as independent [128,1024] tiles loaded over both
DMA paths (HWDGE via sync + SWDGE via gpsimd) in an order that lets
chunk-0 mm1 start after ~2MB of DMA instead of the full preload.

fp8 fraction: the kernel is tensor-engine bound at the bf16 roofline
(216ns per 512-wide matmul back-to-back, ~2.37GHz effective), so the
last 8 of mm2's 32 contraction planes run as 4 fp8e4 DoubleRow pairs
(2 planes/instr at 2x MAC rate). fp8 needs w2*512 to stay out of e4m3
subnormals, so those planes accumulate in a separate PSUM tile that a
fused DVE op merges back: out = (psum_q * 1/512) + psum_bf16 (+b2).
Measured end-to-end rel err 1.90e-2 vs the 2e-2 gate (deterministic
fp8 quantization noise on 25% of mm2) for ~6% less tensor-engine time.
All matmuls accumulate fp32 in PSUM; bf16/fp8 only round the inputs.
Other measured details: PE p-state ramps 0.65->2.4GHz over ~3us, so 30
dummy matmuls warm it during the ~13.6us prologue DMA window; the
prologue is bounded by DMA queue spin-up (HWDGE 2.5-5.7us, SWDGE
~8.7us) + the 3MB chunk-0 critical mass at ~52GB/s/queue.
"""

import numpy as np
import ml_dtypes

import concourse.bacc as bacc
import concourse.bass as bass
import concourse.mybir as mybir
import concourse.tile as tile
from concourse import bass_utils

P = 128
D = 1024
H = 4096
NTOK = 4096  # B*N per expert
NCORES = 8
CHUNK = 512  # tokens per pipeline chunk
NCHUNK = NTOK // CHUNK
F32 = mybir.dt.float32
BF16 = mybir.dt.bfloat16
F8 = mybir.dt.float8e4
GELU = mybir.ActivationFunctionType.Gelu
DR = mybir.MatmulPerfMode.DoubleRow
MULT = mybir.AluOpType.mult
ADD = mybir.AluOpType.add

DK = D // P    # 8   k-tiles of D
HM = H // P    # 32  tiles of H
HQ = 4         # W1 loaded in 4 column quarters (1024 wide)
TSUB = CHUNK // P  # 4 token subtiles per chunk
DC = D // 512  # 2   512-wide output column chunks

NQP = 4              # fp8 DoubleRow pairs in mm2 (planes QBASE..HM-1)
QBASE = HM - 2 * NQP  # first fp8 plane (26)
W2SCALE = 512.0      # host pre-scale on fp8 w2 (keeps e4m3 out of subnormals)


def build_program(act=GELU):
    nc = bacc.Bacc("TRN2", target_bir_lowering=False, debug=False,
                   num_devices=NCORES)

    xt_d = nc.dram_tensor("xt", (D, NTOK), BF16, kind="ExternalInput").ap()
    w1 = nc.dram_tensor("w1", (D, H), BF16, kind="ExternalInput").ap()
    # biases arrive pre-arranged from the host: b1 as [128, 32] (H on
    # partitions), b2 replicated to [128, 1024] — plain contiguous DMAs
    b1 = nc.dram_tensor("b1", (P, HM), F32, kind="ExternalInput").ap()
    w2 = nc.dram_tensor("w2", (QBASE * P, D), BF16, kind="ExternalInput").ap()
    w2q = nc.dram_tensor("w2q", (NQP, P, 2, D), F8, kind="ExternalInput").ap()
    b2 = nc.dram_tensor("b2", (P, D), F32, kind="ExternalInput").ap()
    out = nc.dram_tensor("out", (NTOK, D), F32, kind="ExternalOutput").ap()

    with tile.TileContext(nc) as tc:
        with (
            tc.tile_pool(name="consts", bufs=1) as consts,
            tc.tile_pool(name="weights", bufs=1) as wpool,
            tc.tile_pool(name="xt", bufs=2) as xtp,
            tc.tile_pool(name="ht", bufs=1) as htp,
            tc.tile_pool(name="outp", bufs=4) as outp,
            tc.tile_pool(name="p1", bufs=2, space="PSUM") as p1p,
            tc.tile_pool(name="p2", bufs=3, space="PSUM") as p2p,
            tc.tile_pool(name="p2q", bufs=3, space="PSUM") as p2qp,
        ):
            def load_xt_chunk(c):
                xt = [xtp.tile([P, CHUNK], BF16, tag=f"xt{dk}",
                               name=f"xt{dk}_c{c}") for dk in range(DK)]
                for dk in range(DK):
                    eng = nc.sync if dk % 2 == 0 else nc.gpsimd
                    eng.dma_start(
                        xt[dk], xt_d[dk * P:(dk + 1) * P,
                                     c * CHUNK:(c + 1) * CHUNK])
                return xt

            # b1 first — tiny, and the first gelu (which drains the p1
            # PSUM pool) needs it
            b1_sb = consts.tile([P, HM], F32, tag="b1")
            nc.sync.dma_start(b1_sb, b1)

            # PE p-state warmup: the tensor engine ramps 0.65 -> 1.2 ->
            # 2.4GHz over ~3us of continuous work, and real work can't
            # start until ~13.6us of prologue DMA anyway. Burn the idle
            # window on dummy matmuls over a memset tile so the real
            # stream opens at full clock. ~30 instrs ≈ 10us with ramp.
            warm = consts.tile([P, 512], BF16, tag="warm")
            nc.vector.memset(warm, 0)
            for i in range(30):
                pw = p1p.tile([P, CHUNK], F32, tag="p1", name=f"warm{i}")
                nc.tensor.matmul(pw, warm[:, 0:P], warm,
                                 start=True, stop=True)

            # chunk-0 activations interleaved dk-major with the first W1
            # column-quarter so mm1's k-loop unblocks progressively.
            # (Measured: the ~13.6us prologue is bounded by DMA queue
            # spin-up (2.5-8.7us staggered) + the 3MB critical mass at
            # ~52GB/s per queue; splitting tiles into finer strips only
            # floods descriptors and made it worse.)
            w1t = [[None] * HQ for _ in range(DK)]

            def alloc_w1(dk, hq):
                t = wpool.tile([P, D], BF16, tag=f"w1_{dk}_{hq}",
                               name=f"w1_{dk}_{hq}")
                w1t[dk][hq] = t
                return t

            # chunk-0 critical stream (xt0 + w1 quarter 0, 3MB) split
            # across BOTH DMA engines: aggregate 16-queue bandwidth
            # beats the SWDGE queue-open latency; all-sync variants
            # starved chunk 0 (measured).
            xt0 = [xtp.tile([P, CHUNK], BF16, tag=f"xt{dk}",
                            name=f"xt{dk}_c0") for dk in range(DK)]
            for dk in range(DK):
                e0 = nc.sync if dk % 2 == 0 else nc.gpsimd
                e1 = nc.gpsimd if dk % 2 == 0 else nc.sync
                e0.dma_start(xt0[dk], xt_d[dk * P:(dk + 1) * P, 0:CHUNK])
                e1.dma_start(alloc_w1(dk, 0), w1[dk * P:(dk + 1) * P, 0:D])

            # quarter 1 (needed at hm=8, ~27us) on HWDGE/sync — the
            # SWDGE backlog made it the laggard; quarters 2-3 balance
            for dk in range(DK):
                nc.sync.dma_start(alloc_w1(dk, 1),
                                  w1[dk * P:(dk + 1) * P, D:2 * D])
            for hq in range(2, HQ):
                for dk in range(DK):
                    eng = nc.sync if (hq * DK + dk) % 2 == 0 else nc.gpsimd
                    eng.dma_start(alloc_w1(dk, hq),
                                  w1[dk * P:(dk + 1) * P,
                                     hq * D:(hq + 1) * D])

            # b2 (replicated on host) — needed from the first out tile ~95us
            b2_rep = consts.tile([P, D], F32, tag="b2rep")
            nc.gpsimd.dma_start(b2_rep, b2)

            w2t = []
            for hk in range(QBASE):
                t = wpool.tile([P, D], BF16, tag=f"w2_{hk}",
                               name=f"w2_{hk}")
                w2t.append(t)
                eng = nc.sync if hk % 2 == 0 else nc.gpsimd
                eng.dma_start(t, w2[hk * P:(hk + 1) * P, :])

            # fp8 w2 pair tiles [128, 2, 1024] for the DoubleRow planes
            w2qt = []
            for j in range(NQP):
                t = wpool.tile([P, 2, D], F8, tag=f"w2q_{j}",
                               name=f"w2q_{j}")
                w2qt.append(t)
                eng = nc.sync if j % 2 == 0 else nc.gpsimd
                eng.dma_start(t, w2q[j])

            # ---- main pipeline over token chunks ----
            for c in range(NCHUNK):
                # xT tiles straight from DRAM (bf16), double-buffered so
                # chunk c+1 prefetches during chunk c.
                xt = xt0 if c == 0 else load_xt_chunk(c)

                # mm1 + gelu -> hT tiles; bf16 for planes < QBASE, fp8
                # pair tiles [128, 2, CHUNK] for the DoubleRow planes
                ht = [htp.tile([P, CHUNK], BF16, tag=f"ht{hm}",
                               name=f"ht{hm}_c{c}") for hm in range(QBASE)]
                hq = [htp.tile([P, 2, CHUNK], F8, tag=f"hq{j}",
                               name=f"hq{j}_c{c}") for j in range(NQP)]
                for hm in range(HM):
                    p1 = p1p.tile([P, CHUNK], F32, tag="p1",
                                  name=f"p1_c{c}h{hm}")
                    hcol = (hm % (HM // HQ)) * P
                    for dk in range(DK):
                        nc.tensor.matmul(
                            p1,
                            w1t[dk][hm // (HM // HQ)][:, hcol:hcol + P],
                            xt[dk],
                            start=(dk == 0), stop=(dk == DK - 1))
                    if hm < QBASE:
                        dst = ht[hm]
                    else:
                        j, i = divmod(hm - QBASE, 2)
                        dst = hq[j][:, i, :]
                    nc.scalar.activation(dst, p1, act,
                                         bias=b1_sb[:, hm:hm + 1], scale=1.0)

                # mm2 (+b2) -> out
                for ts in range(TSUB):
                    p2s = [p2p.tile([P, 512], F32, tag="p2",
                                    name=f"p2_c{c}t{ts}d{dc}")
                           for dc in range(DC)]
                    for hk in range(QBASE):
                        lhsT = ht[hk][:, ts * P:(ts + 1) * P]
                        for dc in range(DC):
                            nc.tensor.matmul(
                                p2s[dc], lhsT,
                                w2t[hk][:, dc * 512:(dc + 1) * 512],
                                start=(hk == 0), stop=(hk == QBASE - 1))
                    # fp8 DoubleRow planes -> separate (scaled) PSUM tiles
                    p2q = [p2qp.tile([P, 512], F32, tag="p2q",
                                     name=f"p2q_c{c}t{ts}d{dc}")
                           for dc in range(DC)]
                    for dc in range(DC):
                        for half in range(2):
                            col = dc * 512 + half * 256
                            for j in range(NQP):
                                nc.tensor.matmul(
                                    p2q[dc][:, half * 256:(half + 1) * 256],
                                    hq[j][:, :, ts * P:(ts + 1) * P],
                                    w2qt[j][:, :, col:col + 256],
                                    start=(j == 0), stop=(j == NQP - 1),
                                    perf_mode=DR)
                    r0 = c * CHUNK + ts * P
                    for dc in range(DC):
                        ot = outp.tile([P, 512], F32, tag="ot",
                                       name=f"ot_c{c}t{ts}d{dc}")
                        nc.vector.tensor_add(
                            ot, p2s[dc], b2_rep[:, dc * 512:(dc + 1) * 512])
                        otq = outp.tile([P, 512], F32, tag="otq",
                                        name=f"otq_c{c}t{ts}d{dc}")
                        nc.vector.scalar_tensor_tensor(
                            otq, p2q[dc], 1.0 / W2SCALE, ot, MULT, ADD)
                        oeng = nc.sync if (ts + dc) % 2 == 0 else nc.gpsimd
                        oeng.dma_start(
                            out[r0:r0 + P, dc * 512:(dc + 1) * 512], otq)

    nc.compile()
    return nc


_CACHE: dict = {}


def _program():
    if "nc" not in _CACHE:
        _CACHE["nc"] = build_program()
    return _CACHE["nc"]


def _in_maps(x, w1, b1, w2, b2):
    x = np.asarray(x, dtype=np.float32)
    w1 = np.asarray(w1, dtype=np.float32)
    b1 = np.asarray(b1, dtype=np.float32)
    w2 = np.asarray(w2, dtype=np.float32)
    b2 = np.asarray(b2, dtype=np.float32)
    bf = ml_dtypes.bfloat16
    f8 = ml_dtypes.float8_e4m3fn
    maps = []
    for e in range(NCORES):
        xt_e = np.ascontiguousarray(
            x[:, e].reshape(NTOK, D).T.astype(bf))  # [D, NTOK] bf16
        w2_e = w2[e]
        # fp8 pair layout: w2q[j, p, i, d] = S * w2[(QBASE+2j+i)*128+p, d]
        w2_tail = w2_e[QBASE * P:].reshape(NQP, 2, P, D).transpose(0, 2, 1, 3)
        maps.append({
            "xt": xt_e,
            "w1": np.ascontiguousarray(w1[e].astype(bf)),
            "b1": np.ascontiguousarray(b1[e].reshape(HM, P).T),
            "w2": np.ascontiguousarray(w2_e[:QBASE * P].astype(bf)),
            "w2q": np.ascontiguousarray(
                (w2_tail * W2SCALE).astype(f8)),
            "b2": np.ascontiguousarray(
                np.broadcast_to(b2[e], (P, D))),
        })
    return maps


def _install_ntff_hook_shim():
    """Provide antenv.axon_hooks if the image lacks it, wiring the NTFF
    profile hook straight to libaxon_pjrt.so (mirrors trn_agent_boot)."""
    import sys
    try:
        from antenv.axon_hooks import get_axon_ntff_profile_hook  # noqa: F401
        return
    except ImportError:
        pass
    import contextlib
    import ctypes
    import types

    import antenv

    hook = None
    so_path = "/opt/axon/libaxon_pjrt.so"
    try:
        lib = ctypes.CDLL(so_path)
        if hasattr(lib, "axon_start_nrt_profile"):
            lib.axon_start_nrt_profile.argtypes = [
                ctypes.POINTER(ctypes.c_int64), ctypes.c_size_t]
            lib.axon_start_nrt_profile.restype = ctypes.c_int64
            lib.axon_stop_nrt_profile.argtypes = [ctypes.c_char_p]
            lib.axon_stop_nrt_profile.restype = ctypes.c_int64

            @contextlib.contextmanager
            def _hook(output_dir, device_ids):
                import jax
                jax.devices()
                if device_ids:
                    ids = (ctypes.c_int64 * len(device_ids))(*device_ids)
                    rc = lib.axon_start_nrt_profile(ids, len(device_ids))
                else:
                    rc = lib.axon_start_nrt_profile(None, 0)
                if rc != 0:
                    raise RuntimeError(f"axon_start_nrt_profile rc={rc}")
                try:
                    yield
                finally:
                    n = lib.axon_stop_nrt_profile(str(output_dir).encode())
                    print(f"ntff profile: {n} file(s) -> {output_dir}")

            hook = _hook
    except OSError:
        pass

    mod = types.ModuleType("antenv.axon_hooks")
    mod._hook = hook
    mod.get_axon_ntff_profile_hook = lambda: mod._hook
    mod.set_axon_ntff_profile_hook = lambda h: setattr(mod, "_hook", h)
    sys.modules["antenv.axon_hooks"] = mod
    antenv.axon_hooks = mod


def run_spmd(x, w1, b1, w2, b2, trace=False):
    if trace:
        _install_ntff_hook_shim()
    nc = _program()
    res = bass_utils.run_bass_kernel_spmd(
        nc, _in_maps(x, w1, b1, w2, b2), core_ids=list(range(NCORES)),
        trace=trace)
    outs = [r["out"].reshape(4, 1024, D) for r in res.results]
    full = np.stack(outs, axis=1).astype(np.float32)  # [4, 8, 1024, 1024]
    return full, res


def kernel(x, w1, b1, w2, b2):
    full, _ = run_spmd(x, w1, b1, w2, b2)
    return full


# revision 19
# speedup vs baseline: 1.0689x; 1.0105x over previous
"""Expert-parallel MoE MLP kernel for TRN2 (8 NeuronCores, 1 expert/core).

Math per core (expert e):
    h   = gelu(x_e @ w1_e + b1_e)      x_e: [4096, 1024], w1_e: [1024, 4096]
    out = h @ w2_e + b2_e              w2_e: [4096, 1024]

Host-side prep (inside kernel(), part of the sharding step): x_e is
transposed to [D, tok] and cast to bf16, w1/w2 are cast to bf16. On
device the kernel is then pure matmul work:
  - mm1: stationary = w1 tile [128(D), 128(H)], moving = xT tile
    [128(D), 512(tok)] -> PSUM hT tile; ACT applies exact-erf GELU (+b1
    as per-partition bias) PSUM->SBUF bf16.
  - mm2: stationary = hT tile [128(H), 128(tok)], moving = w2 tile
    [128(H), 512(D)] -> PSUM out tile (fp32); DVE adds b2, DMA out.
Weights live in SBUF as independent [128,1024] tiles loaded over both
DMA paths (HWDGE via sync + SWDGE via gpsimd) in an order that lets
chunk-0 mm1 start after ~2MB of DMA instead of the full preload.

fp8 fraction: the kernel is tensor-engine bound at the bf16 roofline
(216ns per 512-wide matmul back-to-back, ~2.37GHz effective), so the
last 8 of mm2's 32 contraction planes run as 4 fp8e4 DoubleRow pairs
(2 planes/instr at 2x MAC rate). fp8 needs w2*512 to stay out of e4m3
subnormals, so those planes accumulate in a separate PSUM tile that a
fused DVE op merges back: out = (psum_q * 1/512) + psum_bf16 (+b2).
Measured end-to-end rel err 1.90e-2 vs the 2e-2 gate (deterministic
fp8 quantization noise on 25% of mm2) for ~6% less tensor-engine time.
All matmuls accumulate fp32 in PSUM; bf16/fp8 only round the inputs.
Other measured details: PE p-state ramps 0.65->2.4GHz over ~3us, so 30
dummy matmuls warm it during the ~13.6us prologue DMA window; the
prologue is bounded by DMA queue spin-up (HWDGE 2.5-5.7us, SWDGE
~8.7us) + the 3MB chunk-0 critical mass at ~52GB/s/queue.
"""

import numpy as np
import ml_dtypes

import concourse.bacc as bacc
import concourse.bass as bass
import concourse.mybir as mybir
import concourse.tile as tile
from concourse import bass_utils

P = 128
D = 1024
H = 4096
NTOK = 4096  # B*N per expert
NCORES = 8
CHUNK = 512  # tokens per pipeline chunk
NCHUNK = NTOK // CHUNK
F32 = mybir.dt.float32
BF16 = mybir.dt.bfloat16
F8 = mybir.dt.float8e4
GELU = mybir.ActivationFunctionType.Gelu
DR = mybir.MatmulPerfMode.DoubleRow
MULT = mybir.AluOpType.mult
ADD = mybir.AluOpType.add

DK = D // P    # 8   k-tiles of D
HM = H // P    # 32  tiles of H
HQ = 4         # W1 loaded in 4 column quarters (1024 wide)
TSUB = CHUNK // P  # 4 token subtiles per chunk
DC = D // 512  # 2   512-wide output column chunks

NQP = 4              # fp8 DoubleRow pairs in mm2 (planes QBASE..HM-1)
QBASE = HM - 2 * NQP  # first fp8 plane (26)
W2SCALE = 512.0      # host pre-scale on fp8 w2 (keeps e4m3 out of subnormals)


def build_program(act=GELU):
    nc = bacc.Bacc("TRN2", target_bir_lowering=False, debug=False,
                   num_devices=NCORES)

    xt_d = nc.dram_tensor("xt", (D, NTOK), BF16, kind="ExternalInput").ap()
    w1 = nc.dram_tensor("w1", (D, H), BF16, kind="ExternalInput").ap()
    # biases arrive pre-arranged from the host: b1 as [128, 32] (H on
    # partitions), b2 replicated to [128, 1024] — plain contiguous DMAs
    b1 = nc.dram_tensor("b1", (P, HM), F32, kind="ExternalInput").ap()
    w2 = nc.dram_tensor("w2", (QBASE * P, D), BF16, kind="ExternalInput").ap()
    w2q = nc.dram_tensor("w2q", (NQP, P, 2, D), F8, kind="ExternalInput").ap()
    b2 = nc.dram_tensor("b2", (P, D), F32, kind="ExternalInput").ap()
    out = nc.dram_tensor("out", (NTOK, D), F32, kind="ExternalOutput").ap()

    with tile.TileContext(nc) as tc:
        with (
            tc.tile_pool(name="consts", bufs=1) as consts,
            tc.tile_pool(name="weights", bufs=1) as wpool,
            tc.tile_pool(name="xt", bufs=2) as xtp,
            tc.tile_pool(name="ht", bufs=1) as htp,
            tc.tile_pool(name="outp", bufs=4) as outp,
            tc.tile_pool(name="p1", bufs=2, space="PSUM") as p1p,
            tc.tile_pool(name="p2", bufs=3, space="PSUM") as p2p,
            tc.tile_pool(name="p2q", bufs=3, space="PSUM") as p2qp,
        ):
            # steady-state DMAs (xt chunks, out writes) ride HWDGE/sync
            # only (~29GB/s average, trivial for 8 queues): the gpsimd
            # engine's last DMA then lands with the weight prologue, so
            # the slow SWDGE teardown DRAIN overlaps compute instead of
            # extending the epilogue.
            def load_xt_chunk(c):
                xt = [xtp.tile([P, CHUNK], BF16, tag=f"xt{dk}",
                               name=f"xt{dk}_c{c}") for dk in range(DK)]
                for dk in range(DK):
                    nc.sync.dma_start(
                        xt[dk], xt_d[dk * P:(dk + 1) * P,
                                     c * CHUNK:(c + 1) * CHUNK])
                return xt

            # b1 first — tiny, and the first gelu (which drains the p1
            # PSUM pool) needs it
            b1_sb = consts.tile([P, HM], F32, tag="b1")
            nc.sync.dma_start(b1_sb, b1)

            # PE p-state warmup: the tensor engine ramps 0.65 -> 1.2 ->
            # 2.4GHz over ~3us of continuous work, and real work can't
            # start until ~13.6us of prologue DMA anyway. Burn the idle
            # window on dummy matmuls over a memset tile so the real
            # stream opens at full clock. ~30 instrs ≈ 10us with ramp.
            warm = consts.tile([P, 512], BF16, tag="warm")
            nc.vector.memset(warm, 0)
            for i in range(30):
                pw = p1p.tile([P, CHUNK], F32, tag="p1", name=f"warm{i}")
                nc.tensor.matmul(pw, warm[:, 0:P], warm,
                                 start=True, stop=True)

            # chunk-0 activations interleaved dk-major with the first W1
            # column-quarter so mm1's k-loop unblocks progressively.
            # (Measured: the ~13.6us prologue is bounded by DMA queue
            # spin-up (2.5-8.7us staggered) + the 3MB critical mass at
            # ~52GB/s per queue; splitting tiles into finer strips only
            # floods descriptors and made it worse.)
            w1t = [[None] * HQ for _ in range(DK)]

            def alloc_w1(dk, hq):
                t = wpool.tile([P, D], BF16, tag=f"w1_{dk}_{hq}",
                               name=f"w1_{dk}_{hq}")
                w1t[dk][hq] = t
                return t

            # chunk-0 critical stream (xt0 + w1 quarter 0, 3MB) split
            # across BOTH DMA engines: aggregate 16-queue bandwidth
            # beats the SWDGE queue-open latency; all-sync variants
            # starved chunk 0 (measured).
            xt0 = [xtp.tile([P, CHUNK], BF16, tag=f"xt{dk}",
                            name=f"xt{dk}_c0") for dk in range(DK)]
            for dk in range(DK):
                e0 = nc.sync if dk % 2 == 0 else nc.gpsimd
                e1 = nc.gpsimd if dk % 2 == 0 else nc.sync
                e0.dma_start(xt0[dk], xt_d[dk * P:(dk + 1) * P, 0:CHUNK])
                e1.dma_start(alloc_w1(dk, 0), w1[dk * P:(dk + 1) * P, 0:D])

            # quarter 1 (needed at hm=8, ~27us) on HWDGE/sync — the
            # SWDGE backlog made it the laggard; quarters 2-3 balance
            for dk in range(DK):
                nc.sync.dma_start(alloc_w1(dk, 1),
                                  w1[dk * P:(dk + 1) * P, D:2 * D])
            for hq in range(2, HQ):
                for dk in range(DK):
                    eng = nc.sync if (hq * DK + dk) % 2 == 0 else nc.gpsimd
                    eng.dma_start(alloc_w1(dk, hq),
                                  w1[dk * P:(dk + 1) * P,
                                     hq * D:(hq + 1) * D])

            # b2 (replicated on host) — needed from the first out tile ~95us
            b2_rep = consts.tile([P, D], F32, tag="b2rep")
            nc.gpsimd.dma_start(b2_rep, b2)

            w2t = []
            for hk in range(QBASE):
                t = wpool.tile([P, D], BF16, tag=f"w2_{hk}",
                               name=f"w2_{hk}")
                w2t.append(t)
                eng = nc.sync if hk % 2 == 0 else nc.gpsimd
                eng.dma_start(t, w2[hk * P:(hk + 1) * P, :])

            # fp8 w2 pair tiles [128, 2, 1024] for the DoubleRow planes
            w2qt = []
            for j in range(NQP):
                t = wpool.tile([P, 2, D], F8, tag=f"w2q_{j}",
                               name=f"w2q_{j}")
                w2qt.append(t)
                eng = nc.sync if j % 2 == 0 else nc.gpsimd
                eng.dma_start(t, w2q[j])

            # ---- main pipeline over token chunks ----
            for c in range(NCHUNK):
                # xT tiles straight from DRAM (bf16), double-buffered so
                # chunk c+1 prefetches during chunk c.
                xt = xt0 if c == 0 else load_xt_chunk(c)

                # mm1 + gelu -> hT tiles; bf16 for planes < QBASE, fp8
                # pair tiles [128, 2, CHUNK] for the DoubleRow planes
                ht = [htp.tile([P, CHUNK], BF16, tag=f"ht{hm}",
                               name=f"ht{hm}_c{c}") for hm in range(QBASE)]
                hq = [htp.tile([P, 2, CHUNK], F8, tag=f"hq{j}",
                               name=f"hq{j}_c{c}") for j in range(NQP)]
                for hm in range(HM):
                    p1 = p1p.tile([P, CHUNK], F32, tag="p1",
                                  name=f"p1_c{c}h{hm}")
                    hcol = (hm % (HM // HQ)) * P
                    for dk in range(DK):
                        nc.tensor.matmul(
                            p1,
                            w1t[dk][hm // (HM // HQ)][:, hcol:hcol + P],
                            xt[dk],
                            start=(dk == 0), stop=(dk == DK - 1))
                    if hm < QBASE:
                        dst = ht[hm]
                    else:
                        j, i = divmod(hm - QBASE, 2)
                        dst = hq[j][:, i, :]
                    nc.scalar.activation(dst, p1, act,
                                         bias=b1_sb[:, hm:hm + 1], scale=1.0)

                # mm2 (+b2) -> out
                for ts in range(TSUB):
                    p2s = [p2p.tile([P, 512], F32, tag="p2",
                                    name=f"p2_c{c}t{ts}d{dc}")
                           for dc in range(DC)]
                    for hk in range(QBASE):
                        lhsT = ht[hk][:, ts * P:(ts + 1) * P]
                        for dc in range(DC):
                            nc.tensor.matmul(
                                p2s[dc], lhsT,
                                w2t[hk][:, dc * 512:(dc + 1) * 512],
                                start=(hk == 0), stop=(hk == QBASE - 1))
                    # fp8 DoubleRow planes -> separate (scaled) PSUM tiles
                    p2q = [p2qp.tile([P, 512], F32, tag="p2q",
                                     name=f"p2q_c{c}t{ts}d{dc}")
                           for dc in range(DC)]
                    for dc in range(DC):
                        for half in range(2):
                            col = dc * 512 + half * 256
                            for j in range(NQP):
                                nc.tensor.matmul(
                                    p2q[dc][:, half * 256:(half + 1) * 256],
                                    hq[j][:, :, ts * P:(ts + 1) * P],
                                    w2qt[j][:, :, col:col + 256],
                                    start=(j == 0), stop=(j == NQP - 1),
                                    perf_mode=DR)
                    r0 = c * CHUNK + ts * P
                    for dc in range(DC):
                        ot = outp.tile([P, 512], F32, tag="ot",
                                       name=f"ot_c{c}t{ts}d{dc}")
                        nc.vector.tensor_add(
                            ot, p2s[dc], b2_rep[:, dc * 512:(dc + 1) * 512])
                        otq = outp.tile([P, 512], F32, tag="otq",
                                        name=f"otq_c{c}t{ts}d{dc}")
                        nc.vector.scalar_tensor_tensor(
                            otq, p2q[dc], 1.0 / W2SCALE, ot, MULT, ADD)
                        nc.sync.dma_start(
                            out[r0:r0 + P, dc * 512:(dc + 1) * 512], otq)

    nc.compile()
    return nc


_CACHE: dict = {}


def _program():
    if "nc" not in _CACHE:
        _CACHE["nc"] = build_program()
    return _CACHE["nc"]


def _in_maps(x, w1, b1, w2, b2):
    x = np.asarray(x, dtype=np.float32)
    w1 = np.asarray(w1, dtype=np.float32)
    b1 = np.asarray(b1, dtype=np.float32)
    w2 = np.asarray(w2, dtype=np.float32)
    b2 = np.asarray(b2, dtype=np.float32)
    bf = ml_dtypes.bfloat16
    f8 = ml_dtypes.float8_e4m3fn
    maps = []
    for e in range(NCORES):
        xt_e = np.ascontiguousarray(
            x[:, e].reshape(NTOK, D).T.astype(bf))  # [D, NTOK] bf16
        w2_e = w2[e]
        # fp8 pair layout: w2q[j, p, i, d] = S * w2[(QBASE+2j+i)*128+p, d]
        w2_tail = w2_e[QBASE * P:].reshape(NQP, 2, P, D).transpose(0, 2, 1, 3)
        maps.append({
            "xt": xt_e,
            "w1": np.ascontiguousarray(w1[e].astype(bf)),
            "b1": np.ascontiguousarray(b1[e].reshape(HM, P).T),
            "w2": np.ascontiguousarray(w2_e[:QBASE * P].astype(bf)),
            "w2q": np.ascontiguousarray(
                (w2_tail * W2SCALE).astype(f8)),
            "b2": np.ascontiguousarray(
                np.broadcast_to(b2[e], (P, D))),
        })
    return maps


def _install_ntff_hook_shim():
    """Provide antenv.axon_hooks if the image lacks it, wiring the NTFF
    profile hook straight to libaxon_pjrt.so (mirrors trn_agent_boot)."""
    import sys
    try:
        from antenv.axon_hooks import get_axon_ntff_profile_hook  # noqa: F401
        return
    except ImportError:
        pass
    import contextlib
    import ctypes
    import types

    import antenv

    hook = None
    so_path = "/opt/axon/libaxon_pjrt.so"
    try:
        lib = ctypes.CDLL(so_path)
        if hasattr(lib, "axon_start_nrt_profile"):
            lib.axon_start_nrt_profile.argtypes = [
                ctypes.POINTER(ctypes.c_int64), ctypes.c_size_t]
            lib.axon_start_nrt_profile.restype = ctypes.c_int64
            lib.axon_stop_nrt_profile.argtypes = [ctypes.c_char_p]
            lib.axon_stop_nrt_profile.restype = ctypes.c_int64

            @contextlib.contextmanager
            def _hook(output_dir, device_ids):
                import jax
                jax.devices()
                if device_ids:
                    ids = (ctypes.c_int64 * len(device_ids))(*device_ids)
                    rc = lib.axon_start_nrt_profile(ids, len(device_ids))
                else:
                    rc = lib.axon_start_nrt_profile(None, 0)
                if rc != 0:
                    raise RuntimeError(f"axon_start_nrt_profile rc={rc}")
                try:
                    yield
                finally:
                    n = lib.axon_stop_nrt_profile(str(output_dir).encode())
                    print(f"ntff profile: {n} file(s) -> {output_dir}")

            hook = _hook
    except OSError:
        pass

    mod = types.ModuleType("antenv.axon_hooks")
    mod._hook = hook
    mod.get_axon_ntff_profile_hook = lambda: mod._hook
    mod.set_axon_ntff_profile_hook = lambda h: setattr(mod, "_hook", h)
    sys.modules["antenv.axon_hooks"] = mod
    antenv.axon_hooks = mod


def run_spmd(x, w1, b1, w2, b2, trace=False):
    if trace:
        _install_ntff_hook_shim()
    nc = _program()
    res = bass_utils.run_bass_kernel_spmd(
        nc, _in_maps(x, w1, b1, w2, b2), core_ids=list(range(NCORES)),
        trace=trace)
    outs = [r["out"].reshape(4, 1024, D) for r in res.results]
    full = np.stack(outs, axis=1).astype(np.float32)  # [4, 8, 1024, 1024]
    return full, res


def kernel(x, w1, b1, w2, b2):
    full, _ = run_spmd(x, w1, b1, w2, b2)
    return full
